# revision 1
# baseline (speedup 1.0000x reference)
"""Trainium2 Bass kernel for nn_NumDualDescriptorAB.

Reference computation:
    agg[b,w]   = mean(seq[b, w:w+8, :], axis=0)          (sliding window, Nw = S-7)
    y[b,w]     = agg[b,w] @ M.T
    Nk[w]      = Acoeff[:, w%L] * Bbasis[w%L, :]
    D          = mean((y - Nk)^2)

Algebraic decomposition:
    count = B*Nw*m
    t1 = sum_{b,w} agg MtM agg^T = <M^T M, G>_F   with G = sum agg^T agg   (m x m)
    t2 = sum_{b,w} y . Nk = sum_s seqsum[s] . P[s]   with seqsum = sum_b seq[b],
         P = W^T (Nk M)  -- LINEAR in seq, so it is a trivial host reduction.
    t3 = B * ||Nk||^2
    D  = (t1 - 2 t2 + t3) / count

Only the quadratic term G needs the device.  Each of the 8 cores handles 4
batches: sliding windows come from a banded constant matrix W (lhsT, fp8)
applied to 128-row fp8 seq chunks (121 windows per chunk, 17 chunks), the
agg chunk is cast PSUM->SBUF fp8e4m3 (split between DVE and ACT), and
per-batch fp8 Gram matmuls accumulate G (f32 PSUM) in a PSUM bank.  fp8
halves the DMA bytes and quadruples the FWL weight-load rate; the induced
bias on D is ~1.5e-3, well inside the 2e-2 gate (t2/t3 stay exact on host).

Schedule notes (from NTFF traces):
  - The PE HAM clock gate needs ~3.4us of sustained activity before the PE
    un-throttles from 1.2 to 2.4 GHz; 1-column dummy matmuls with no data
    dependencies keep the PE busy from t~1us while the first DMA piece is
    in flight (HBM completion receipt alone is ~2us).
  - An early no-dep ACT op pulls the ~1.3us ACT_TABLE_LOAD into the DMA
    fill phase (tile otherwise schedules it behind the first cast's wait).
  - Steady state runs ~440ns/chunk with PE (win 216 + 4x55 gram), DVE cast
    (272 cols) and ACT cast (240 cols) all balanced; PSUM depth 6 with the
    window matmul emitted mid-gram-group keeps PE bubbles out (<0.4us
    total).
  - The G write-back is a fire-and-forget single-packet DMA issued after
    the TileContext (walrus requires sync info -> dead-semaphore
    then_inc); nothing waits on it, so its transfer and HBM write receipt
    hide entirely inside the runtime teardown.
  - Remaining fixed overhead per execution (~10us): runtime-injected
    per-engine semaphore-clear teardown (~7us, absent from the
    compiler-emitted engine binaries) and the first-piece HBM completion
    receipt (~2.2us) -- outside kernel control.

Host side (float64): P/seqsum/t2, t3, M^T M, and the final combine.
"""

import os

# The device run goes through jax's axon/neuron backend; a cpu-only pin
# (used for reference computations elsewhere) would hide the NeuronCores.
if os.environ.get("JAX_PLATFORMS", "").strip() == "cpu":
    del os.environ["JAX_PLATFORMS"]

import numpy as np
import ml_dtypes

B, S, m, L, RANK = 32, 2048, 128, 64, 8
Nw = S - RANK + 1  # 2041
NCORES = 8
BPC = B // NCORES  # batches per core = 4
CH = 121  # windows per chunk (window w needs rows w..w+7, so 121+7=128 rows)
NCH = (Nw + CH - 1) // CH  # 17 chunks
TAILW = Nw - (NCH - 1) * CH  # 105 windows in the last chunk
CW = BPC * m  # free columns per chunk = 512
WCOLS = 2 * m  # wmat columns (wmain | wtail), stored ahead of seq data

BF16 = ml_dtypes.bfloat16
FP8 = ml_dtypes.float8_e4m3

_NC_CACHE = {}

N_DUMMY = 27  # 1-column dummy matmuls to warm the PE HAM clock gate
# seq DMA pieces (chunk counts) as column ranges of the combined
# [wmat | chunks] tensor, all on the sync HWDGE ring in consumption order.
# Piece 0 additionally carries wmat in the SAME dma_start (one completion
# receipt instead of two).  Later pieces are larger: per-partition
# descriptor size grows with piece size and descriptor overhead is what
# caps the single-queue rate (~180 GB/s at 2KB vs ~270 at 5KB).
PIECE_CHUNKS = [2, 3, 4, 4, 4]


def _build_nc():
    import concourse.bacc as bacc
    import concourse.mybir as mybir
    import concourse.tile as tile

    bf = mybir.dt.bfloat16
    f8 = mybir.dt.float8e4
    f32 = mybir.dt.float32

    nc = bacc.Bacc("TRN2", target_bir_lowering=False, debug=False,
                   enable_partition_id=False)

    seq_d = nc.dram_tensor("seq", [128, WCOLS + NCH * CW], f8,
                           kind="ExternalInput")
    out_d = nc.dram_tensor("out", [128, m], f32, kind="ExternalOutput")

    # raw (non-tile) SBUF tensor so the fire-and-forget DMA below has a
    # concrete access pattern
    s_out = nc.alloc_sbuf_tensor("s_out", [128, m], f32)

    with tile.TileContext(nc) as tc:
        with (
            tc.tile_pool(name="const", bufs=1) as cpool,
            tc.tile_pool(name="aggb", bufs=3) as apool,
            tc.tile_pool(name="psa", bufs=6, space="PSUM") as pspool,
            tc.tile_pool(name="psacc", bufs=1, space="PSUM") as accpool,
        ):
            # --- DMA issue (t=0): all pieces on the sync ring, serial issue
            # so arrival order == consumption order.  Piece 0 carries wmat.
            seq_tiles = [None] * NCH  # chunk -> (tile, base col)
            a = 0
            for pc, n in enumerate(PIECE_CHUNKS):
                if pc == 0:
                    # wmat + chunk 0 in a single transfer
                    t = cpool.tile([128, WCOLS + n * CW], f8, tag="seq0",
                                   name="seqp0")
                    nc.sync.dma_start(out=t[:],
                                      in_=seq_d[:, 0:WCOLS + n * CW])
                    s_w = t
                    base = WCOLS
                else:
                    t = cpool.tile([128, n * CW], f8, tag=f"seq{pc}",
                                   name=f"seqp{pc}")
                    nc.sync.dma_start(
                        out=t[:],
                        in_=seq_d[:, WCOLS + a * CW:WCOLS + (a + n) * CW])
                    base = 0
                for c in range(a, a + n):
                    seq_tiles[c] = (t, base + (c - a) * CW)
                a += n

            def seq_ap(c):
                t, o = seq_tiles[c]
                return t[:, o:o + CW]

            G_ps = accpool.tile([128, m], f32, tag="G")
            scr_ps = accpool.tile([128, m], f32, tag="scr")

            # --- PE warmup: no data dependencies, 1-column weights so the
            # LDW path stays free; keeps the HAM activity window busy while
            # the first seq piece is in flight.
            dum = cpool.tile([128, m], f8, tag="dum")
            nc.gpsimd.memset(dum[:], 0)
            # Early no-dep ACT op: forces the ~1.3us ACT_TABLE_LOAD (which
            # tile otherwise schedules behind the first cast's semaphore
            # wait) to run during the DMA fill phase.
            warm = cpool.tile([128, 1], f8, tag="warm")
            nc.scalar.copy(warm[:], dum[:, 0:1])
            for _ in range(N_DUMMY):
                nc.tensor.matmul(scr_ps[0:1, :], dum[:, 0:1], dum[:],
                                 start=True, stop=True, skip_group_check=True)

            # --- main pipeline: win(c) -> cast(c) (DVE half + ACT half)
            # -> 4 gram matmuls
            agg_tiles = {}

            def emit_win(c):
                agg_ps = pspool.tile([128, CW], f32, tag="aggps",
                                     name=f"agg{c}")
                agg_tiles[c] = agg_ps
                wsel = s_w[:, 0:m] if c < NCH - 1 else s_w[:, m:2 * m]
                nc.tensor.matmul(agg_ps[:], wsel, seq_ap(c),
                                 start=True, stop=True)

            # NPRE < PSUM bufs so the win emitted mid-group has its WAR
            # dependency (cast of the buffer it reuses) long satisfied and
            # can fill the PE gap while the ACT cast half finishes.
            NPRE = 5
            for c in range(NPRE):
                emit_win(c)
            for c in range(NCH):
                aggb = apool.tile([128, CW], f8, tag="aggb", name=f"aggb{c}")
                # DVE is a touch faster than ACT per column; balance ~272/240
                half = 272
                nc.vector.tensor_copy(aggb[:, 0:half],
                                      agg_tiles[c][:, 0:half])
                nc.scalar.copy(aggb[:, half:CW], agg_tiles[c][:, half:CW])

                def gram(j):
                    blk = aggb[:, j * m:(j + 1) * m]
                    nc.tensor.matmul(
                        G_ps[:], blk, blk,
                        start=(c == 0 and j == 0),
                        stop=(c == NCH - 1 and j == BPC - 1),
                        skip_group_check=True,
                    )

                gram(0)
                gram(1)
                if c + NPRE < NCH:
                    emit_win(c + NPRE)
                gram(2)
                gram(3)

            nc.vector.tensor_copy(s_out.ap(), G_ps[:])

    # Fire-and-forget output DMA: issued after the TileContext's drain
    # barrier (so the copy above is complete), with a completion semaphore
    # that nothing waits on (walrus requires sync info on DGE ops).  The
    # ~2us HBM write receipt overlaps the fixed per-engine semaphore-clear
    # teardown (~7us) instead of preceding it; the data is long landed by
    # stream end.
    ff_sem = nc.alloc_semaphore("ff_out")
    nc.sync.dma_start(out=out_d[:], in_=s_out.ap(),
                      single_packet=True).then_inc(ff_sem, 16)

    nc.compile()
    return nc


def get_nc():
    if "nc" not in _NC_CACHE:
        _NC_CACHE["nc"] = _build_nc()
    return _NC_CACHE["nc"]


def _chunk_rows():
    rows = CH * np.arange(NCH)[:, None] + np.arange(128)[None, :]  # [NCH, 128]
    valid = rows < S
    return rows, valid


def host_prep(seq_batch, M, Acoeff, Bbasis):
    """Build per-core device inputs + host-side exact terms."""
    rows, valid = _chunk_rows()
    rows_c = np.minimum(rows, S - 1)

    # seq image: per core [128, NCH, BPC, m] with seq_img[p, c, j] = seq[4k+j, 121c+p]
    g = seq_batch[:, rows_c, :].astype(FP8)  # [B, NCH, 128, m]
    g[:, ~valid, :] = 0
    imgs = np.ascontiguousarray(
        g.reshape(NCORES, BPC, NCH, 128, m).transpose(0, 3, 2, 1, 4)
    ).reshape(NCORES, 128, NCH * BPC * m)

    # banded window matrices (lhsT): out[w, n] = sum_k W[k, w] rhs[k, n]
    k = np.arange(128)[:, None]
    w = np.arange(128)[None, :]
    band = ((k - w >= 0) & (k - w < RANK)).astype(np.float32) / RANK
    wmain = band * (w < CH)
    wtail = band * (w < TAILW)
    wmat = np.concatenate([wmain, wtail], axis=1).astype(FP8)  # [128, 256]

    # combined device input: [wmat | seq chunks]
    full = np.concatenate(
        [np.broadcast_to(wmat, (NCORES, 128, WCOLS)), imgs], axis=2)
    full = np.ascontiguousarray(full)

    # linear terms in float64 on host: t2 = <seqsum, P>, t3 = B*||Nk||^2
    M64 = np.asarray(M, np.float64)
    kmod = np.arange(Nw) % L
    Nk = (np.asarray(Acoeff, np.float64).T[kmod]
          * np.asarray(Bbasis, np.float64)[kmod])  # [Nw, m]
    Ntil = Nk @ M64  # [Nw, m]
    csum = np.concatenate([np.zeros((1, m)), np.cumsum(Ntil, axis=0)])
    s = np.arange(S)
    lo = np.maximum(s - (RANK - 1), 0)
    hi = np.minimum(s, Nw - 1)
    P = (csum[hi + 1] - csum[lo]) / RANK  # [S, m]

    seqsum = np.asarray(seq_batch, np.float64).sum(axis=0)  # [S, m]
    t2 = float((seqsum * P).sum())
    t3 = B * float((Nk ** 2).sum())
    MtM = M64.T @ M64
    return full, MtM, t2, t3


def combine(results, MtM, t2, t3):
    """results: list of 8 arrays [128, 128] f32 (per-core G) -> scalar D."""
    G = np.zeros((m, m), np.float64)
    for r in results:
        G += np.asarray(r, np.float64)
    t1 = float((MtM * G).sum())
    D = (t1 - 2.0 * t2 + t3) / (B * Nw * m)
    return np.float32(D)


def kernel(seq_batch, M, Acoeff, Bbasis):
    from concourse.bass_utils import run_bass_kernel_spmd

    seq_batch = np.asarray(seq_batch, np.float32)
    full, MtM, t2, t3 = host_prep(seq_batch, M, Acoeff, Bbasis)

    nc = get_nc()
    in_maps = [{"seq": full[c]} for c in range(NCORES)]
    res = run_bass_kernel_spmd(nc, in_maps, core_ids=list(range(NCORES)))
    outs = [res.results[c]["out"] for c in range(NCORES)]
    return combine(outs, MtM, t2, t3)

